# revision 5
# baseline (speedup 1.0000x reference)
"""v15: v12 + per-head bf16 staging (hbf) in the outer pool scope so its
alloc barrier clears at t=0 and the Pool-queue reads run as soon as the
scratch data exists (they previously waited ~90us for phase-A SBUF to free).
Paid for by moving kT to the bf16 DRAM scratch (net -4KB SBUF). wo loads
emit after head 0's reads.

v12: v9 + two-sweep v-projection: sweep 1 accumulates 8 tk tiles across
all 8 PSUM banks chunk-by-chunk as x arrives (PE saturated during the x
load); sweep 2 finishes tk 8-15 once x is resident.

v9: v8 but per-head attention reads are normal phase-B loads on the idle
Pool queue (v8 emitted them in phase A with a single-buffered pool that
parked the SP queue head on a long semaphore wait - and hit
NRT_EXEC_UNIT_UNRECOVERABLE on hardware).

Original: Trainium2 Bass kernel for LGeM self-attention (b=2, t=2048, c=2048, h=16, d=128).

v8: fp32r matmuls (self-loading weights; bf16 matmuls pay a ~550ns/mm
ldweights penalty on real TRN2; walrus rejects mixed-dtype matmuls and
ldw-opt). All host->device tensors ship bf16; upcasts to fp32r run on DVE
(8-deep exec queue) and ACT, which are idle during the projections.

Schedule/queue design (from timeline-sim gap analysis):
  - pools open before any emission so no engine's first DMA waits on pool
    allocation barriers; wv chunk loads (Pool/SWDGE queue) are emitted
    before the x loads so the first v-proj matmul starts ~3us in,
  - x chunks upcast through per-chunk fp32r tiles (dependency granularity
    = one chunk, not the whole 128KB tile),
  - wq/wk slices load on the ACT queue (idle after the x odd-chunk loads);
    Pool/SWDGE is slow (~1us/DMA engine time) so it only carries wv/wo,
  - qT and v round-trip a bf16 DRAM scratch on the SP queue; the per-head
    attention reads are emitted inside phase A right after their head's
    scratch writes, so they land long before attention needs them,
  - kT stays SBUF-resident bf16; cf factors ship bf16 and feed mixed-dtype
    DVE multiplies (f32 PSUM x bf16 -> bf16),
  - output is written bf16; host sums the mp-partials in fp32.

Numerics: the validated "bf16 everywhere" dataflow (5e-3 max rel err vs fp32
reference; tolerance 2e-2) with fp32r accumulation between stages.

Sharding: 8 cores = 2 (batch) x 4 (head-groups of 4 heads). Per core: q/k/v
projections for its 4 heads, attention, partial output projection (its 512
rows of Wo); host sums the 4 mp-partials per batch.

Math notes (matching the reference exactly):
  - rope is q*(cos+sin) elementwise (the module's rotate_half is identity),
    with the 1/sqrt(t) logit scale folded into cfq.
  - softmax without max-subtraction: logits ~N(0,0.2^2), exp cannot overflow.
  - scores are built transposed, S_T[tk,tq] = kT_tile.T @ qT, so attn@v needs
    no transposes; the softmax denominator comes from a ones[128,128]
    stationary matmul (free partition-broadcast for the reciprocal multiply).
"""

import sys

sys.path.insert(0, "/opt/trn_rl_repo")

import math

import numpy as np
import ml_dtypes

import concourse.bass as bass
import concourse.mybir as mybir
import concourse.tile as tile
from concourse import bacc, bass_utils

F32 = mybir.dt.float32
F32R = mybir.dt.float32r
BF16 = mybir.dt.bfloat16
NP_BF16 = ml_dtypes.bfloat16
Copy = mybir.ActivationFunctionType.Copy
Exp = mybir.ActivationFunctionType.Exp

HIDDEN = 2048
HEADS = 16
HEAD_DIM = 128
SEQ = 2048
BATCH = 2
N_CORES = 8
MP = 4
HG = HEADS // MP
THETA = 10000.0


def build_attention_nc(T, C, HG, D, use_mask=False):
    DG = HG * D  # 512
    CCH = C // 128  # 16
    TQC = min(512, T)
    NTQ = T // TQC  # 4
    NTK = T // 128  # 16
    NQT = T // 128  # 16
    NOC = C // TQC  # 4

    nc = bacc.Bacc("TRN2", target_bir_lowering=False, debug=False)

    xT = nc.dram_tensor("xT", [C, T], BF16, kind="ExternalInput").ap()
    wq = nc.dram_tensor("wq", [C, DG], BF16, kind="ExternalInput").ap()
    wk = nc.dram_tensor("wk", [C, DG], BF16, kind="ExternalInput").ap()
    wv = nc.dram_tensor("wv", [C, DG], BF16, kind="ExternalInput").ap()
    wo = nc.dram_tensor("wo", [DG, C], BF16, kind="ExternalInput").ap()
    cfq = nc.dram_tensor("cfq", [D, T], BF16, kind="ExternalInput").ap()
    cfk = nc.dram_tensor("cfk", [D, T], BF16, kind="ExternalInput").ap()
    if use_mask:
        maskT = nc.dram_tensor("maskT", [T, T], F32, kind="ExternalInput").ap()
    out = nc.dram_tensor("out", [T, C], BF16, kind="ExternalOutput").ap()

    with tile.TileContext(nc) as tc:
        with (
            tc.tile_pool(name="scratch", bufs=1, space="DRAM") as dpool,
            tc.tile_pool(name="hbf", bufs=1) as hbfpool,
        ):
            qT_s = dpool.tile([DG, T], BF16, tag="qTs")
            kT_s = dpool.tile([DG, T], BF16, tag="kTs")
            v_s = dpool.tile([T, DG], BF16, tag="vs")

            # ---------------- Phase A ----------------
            with tc.tile_pool(name="xp", bufs=1) as xpool:
                xT_c = [
                    xpool.tile([128, T], F32R, name=f"xc{cc}", tag=f"xc{cc}")
                    for cc in range(CCH)
                ]
                with tc.tile_pool(name="cf", bufs=1) as cfpool:
                    cfq_sb = cfpool.tile([128, T], BF16, tag="cfq")
                    cfk_sb = cfpool.tile([128, T], BF16, tag="cfk")

                    with (
                        tc.tile_pool(name="xbf", bufs=3) as xbfpool,
                        tc.tile_pool(name="wvbf", bufs=1) as wvbfpool,
                        tc.tile_pool(name="wvf", bufs=1) as wvfpool,
                        tc.tile_pool(name="vst", bufs=2) as vstpool,
                        tc.tile_pool(name="vps", bufs=1, space="PSUM") as vps,
                    ):
                        # wv first on the Pool queue: first chunks land ~2us
                        wv_c = [
                            wvfpool.tile(
                                [128, DG], F32R, name=f"wvc{cc}", tag=f"wvc{cc}"
                            )
                            for cc in range(CCH)
                        ]
                        for cc in range(CCH):
                            wvb = wvbfpool.tile([128, DG], BF16, tag="wvb")
                            nc.gpsimd.dma_start(
                                wvb[:], wv[cc * 128 : (cc + 1) * 128, :]
                            )
                            if cc % 2 == 0:
                                nc.vector.tensor_copy(wv_c[cc][:], wvb[:])
                            else:
                                nc.scalar.activation(wv_c[cc][:], wvb[:], Copy)
                        for cc in range(CCH):
                            xb = xbfpool.tile([128, T], BF16, tag="xb")
                            eng = nc.sync if cc % 2 == 0 else nc.scalar
                            eng.dma_start(xb[:], xT[cc * 128 : (cc + 1) * 128, :])
                            if cc % 2 == 0:
                                nc.vector.tensor_copy(xT_c[cc][:], xb[:])
                            else:
                                nc.scalar.activation(xT_c[cc][:], xb[:], Copy)
                        # cf (bf16) after the x chunks on the SP queue
                        nc.sync.dma_start(cfq_sb[:D, :], cfq)
                        nc.sync.dma_start(cfk_sb[:D, :], cfk)
                        # v-proj sweep 1: 8 tk tiles accumulate in parallel,
                        # chunk-major, so PE has 8 matmuls of work per arriving
                        # x chunk instead of stalling on the full contraction.
                        NSW = 8
                        pvs = [
                            vps.tile([128, DG], F32, name=f"pv{tk}", tag=f"pv{tk}")
                            for tk in range(NSW)
                        ]
                        for cc in range(CCH):
                            for tk in range(NSW):
                                nc.tensor.matmul(
                                    pvs[tk][:],
                                    xT_c[cc][:, tk * 128 : (tk + 1) * 128],
                                    wv_c[cc][:],
                                    start=(cc == 0),
                                    stop=(cc == CCH - 1),
                                )
                        for tk in range(NSW):
                            vt = vstpool.tile([128, DG], BF16, tag="vt")
                            nc.vector.tensor_copy(vt[:], pvs[tk][:])
                            nc.sync.dma_start(v_s[tk * 128 : (tk + 1) * 128, :], vt[:])
                        # sweep 2: remaining tk at full speed (x resident)
                        for tk in range(NSW, NTK):
                            pv = vps.tile([128, DG], F32, name=f"pv{(tk - NSW) % NSW}", tag=f"pv{(tk - NSW) % NSW}")
                            for cc in range(CCH):
                                nc.tensor.matmul(
                                    pv[:],
                                    xT_c[cc][:, tk * 128 : (tk + 1) * 128],
                                    wv_c[cc][:],
                                    start=(cc == 0),
                                    stop=(cc == CCH - 1),
                                )
                            vt = vstpool.tile([128, DG], BF16, tag="vt")
                            nc.vector.tensor_copy(vt[:], pv[:])
                            nc.sync.dma_start(v_s[tk * 128 : (tk + 1) * 128, :], vt[:])

                    # ---- qT/kT ----
                    with (
                        tc.tile_pool(name="wbf", bufs=3) as wbfpool,
                        tc.tile_pool(name="wcc", bufs=2) as wccpool,
                        tc.tile_pool(name="qst", bufs=2) as qstpool,
                        tc.tile_pool(name="qkps", bufs=4, space="PSUM") as qkps,
                    ):
                        for h in range(HG):
                            for wi, (w_in, cf_sb) in enumerate(
                                ((wq, cfq_sb), (wk, cfk_sb))
                            ):
                                wb = wbfpool.tile([128, CCH, D], BF16, tag="wb")
                                for cc in range(CCH):
                                    nc.scalar.dma_start(
                                        wb[:, cc, :],
                                        w_in[
                                            cc * 128 : (cc + 1) * 128,
                                            h * D : (h + 1) * D,
                                        ],
                                    )
                                wc = wccpool.tile([128, CCH, D], F32R, tag="wc")
                                nc.vector.tensor_copy(wc[:], wb[:])
                                for tq in range(NTQ):
                                    pm = qkps.tile([128, TQC], F32)
                                    for cc in range(CCH):
                                        nc.tensor.matmul(
                                            pm[:],
                                            wc[:, cc, :],
                                            xT_c[cc][:, tq * TQC : (tq + 1) * TQC],
                                            start=(cc == 0),
                                            stop=(cc == CCH - 1),
                                        )
                                    dst_s = qT_s if wi == 0 else kT_s
                                    qs = qstpool.tile([128, TQC], BF16, tag="qs")
                                    nc.vector.tensor_mul(
                                        qs[:D, :],
                                        pm[:D, :],
                                        cf_sb[:D, tq * TQC : (tq + 1) * TQC],
                                    )
                                    nc.sync.dma_start(
                                        dst_s[
                                            h * D : (h + 1) * D,
                                            tq * TQC : (tq + 1) * TQC,
                                        ],
                                        qs[:D, :],
                                    )

            # ---------------- Phase B: attention ----------------
            with tc.tile_pool(name="o2", bufs=1) as o2pool:
                out2_sb = o2pool.tile([128, HG, T], F32R)
                with (
                    tc.tile_pool(name="wobf", bufs=2) as wobfpool,
                    tc.tile_pool(name="wop", bufs=1) as wopool,
                ):
                    with (
                        tc.tile_pool(name="hp", bufs=2) as hpool,
                        tc.tile_pool(name="cst", bufs=1) as cstpool,
                        tc.tile_pool(name="ep", bufs=6) as epool,
                        tc.tile_pool(name="mp", bufs=4) as mpool,
                        tc.tile_pool(name="rp", bufs=2) as rpool,
                        tc.tile_pool(name="scps", bufs=4, space="PSUM") as scps,
                        tc.tile_pool(name="o2ps", bufs=2, space="PSUM") as o2ps,
                        tc.tile_pool(name="sps", bufs=2, space="PSUM") as sps,
                    ):
                        ones_f = cstpool.tile([128, 128], F32)
                        nc.vector.memset(ones_f[:], 1.0)
                        ones = cstpool.tile([128, 128], F32R)
                        nc.vector.tensor_copy(ones[:], ones_f[:])
                        wo_sb = wopool.tile([128, HG, C], F32R)

                        def _emit_wo_loads():
                            for hh in range(HG):
                                wob = wobfpool.tile([128, C], BF16, tag="wob")
                                nc.gpsimd.dma_start(
                                    wob[:D, :], wo[hh * D : (hh + 1) * D, :]
                                )
                                if hh % 2 == 0:
                                    nc.vector.tensor_copy(
                                        wo_sb[:D, hh, :], wob[:D, :]
                                    )
                                else:
                                    nc.scalar.activation(
                                        wo_sb[:D, hh, :], wob[:D, :], Copy
                                    )
                        for h in range(HG):
                            if h == 1:
                                _emit_wo_loads()
                            qT_sb = hpool.tile([128, T], F32R, tag="qT")
                            kT_sb = hpool.tile([128, T], F32R, tag="kT")
                            v_sb = hpool.tile([128, NTK, D], F32R, tag="v")
                            qbf = hbfpool.tile([128, T], BF16, tag="qbf")
                            kbf = hbfpool.tile([128, T], BF16, tag="kbf")
                            vbf = hbfpool.tile([128, NTK, D], BF16, tag="vbf")
                            nc.gpsimd.dma_start(qbf[:D, :], qT_s[h * D : (h + 1) * D, :])
                            nc.gpsimd.dma_start(kbf[:D, :], kT_s[h * D : (h + 1) * D, :])
                            for i in range(NTK):
                                nc.gpsimd.dma_start(
                                    vbf[:, i, :],
                                    v_s[i * 128 : (i + 1) * 128, h * D : (h + 1) * D],
                                )
                            nc.vector.tensor_copy(kT_sb[:D, :], kbf[:D, :])
                            nc.vector.tensor_copy(qT_sb[:D, :], qbf[:D, :])
                            nc.vector.tensor_copy(v_sb[:, :, :], vbf[:, :, :])
                            for tq in range(NTQ):
                                o2p = o2ps.tile([128, TQC], F32)
                                sp = sps.tile([128, TQC], F32)
                                pending = []
                                for tk in range(NTK):
                                    scp = scps.tile([128, TQC], F32)
                                    nc.tensor.matmul(
                                        scp[:],
                                        kT_sb[:D, tk * 128 : (tk + 1) * 128],
                                        qT_sb[:D, tq * TQC : (tq + 1) * TQC],
                                        start=True,
                                        stop=True,
                                    )
                                    et = epool.tile([128, TQC], F32R, tag="et")
                                    if use_mask:
                                        mt = mpool.tile([128, TQC], F32, tag="mt")
                                        nc.gpsimd.dma_start(
                                            mt[:],
                                            maskT[
                                                tk * 128 : (tk + 1) * 128,
                                                tq * TQC : (tq + 1) * TQC,
                                            ],
                                        )
                                        ma = mpool.tile([128, TQC], F32, tag="ma")
                                        nc.vector.tensor_add(ma[:], scp[:], mt[:])
                                        nc.scalar.activation(et[:], ma[:], Exp)
                                    else:
                                        nc.scalar.activation(et[:], scp[:], Exp)
                                    pending.append((et, tk))
                                    if len(pending) > 2:
                                        p_et, p_tk = pending.pop(0)
                                        nc.tensor.matmul(
                                            o2p[:],
                                            v_sb[:, p_tk, :],
                                            p_et[:],
                                            start=(p_tk == 0),
                                            stop=False,
                                        )
                                        nc.tensor.matmul(
                                            sp[:],
                                            ones[:],
                                            p_et[:],
                                            start=(p_tk == 0),
                                            stop=False,
                                        )
                                while pending:
                                    p_et, p_tk = pending.pop(0)
                                    nc.tensor.matmul(
                                        o2p[:],
                                        v_sb[:, p_tk, :],
                                        p_et[:],
                                        start=(p_tk == 0),
                                        stop=(p_tk == NTK - 1),
                                    )
                                    nc.tensor.matmul(
                                        sp[:],
                                        ones[:],
                                        p_et[:],
                                        start=(p_tk == 0),
                                        stop=(p_tk == NTK - 1),
                                    )
                                rt = rpool.tile([128, TQC], F32)
                                nc.vector.reciprocal(rt[:], sp[:])
                                nc.vector.tensor_mul(
                                    out2_sb[:D, h, tq * TQC : (tq + 1) * TQC],
                                    o2p[:D, :],
                                    rt[:D, :],
                                )

                    # -------- output projection --------
                    with (
                        tc.tile_pool(name="fst", bufs=4) as fpool,
                        tc.tile_pool(name="fps", bufs=4, space="PSUM") as fps,
                    ):
                        for qt in range(NQT):
                            for oc in range(NOC):
                                fp = fps.tile([128, TQC], F32)
                                for h in range(HG):
                                    nc.tensor.matmul(
                                        fp[:],
                                        out2_sb[:D, h, qt * 128 : (qt + 1) * 128],
                                        wo_sb[:D, h, oc * TQC : (oc + 1) * TQC],
                                        start=(h == 0),
                                        stop=(h == HG - 1),
                                    )
                                ft = fpool.tile([128, TQC], BF16, tag="ft")
                                if oc % 2 == 0:
                                    nc.vector.tensor_copy(ft[:], fp[:])
                                else:
                                    nc.scalar.activation(ft[:], fp[:], Copy)
                                nc.sync.dma_start(
                                    out[
                                        qt * 128 : (qt + 1) * 128,
                                        oc * TQC : (oc + 1) * TQC,
                                    ],
                                    ft[:],
                                )

    nc.compile()
    return nc


def compute_cfacs(T, D, theta=THETA):
    freq = 1.0 / theta ** (np.arange(0, D, 2, dtype=np.float64) / D)
    t = np.arange(T, dtype=np.float64)
    m = np.einsum("i,j->ij", t, freq)
    m = np.concatenate([m, m], axis=-1)
    cfac = (np.cos(m) + np.sin(m)).astype(np.float32)
    cfk = np.ascontiguousarray(cfac.T).astype(NP_BF16)
    cfq = np.ascontiguousarray(cfac.T / np.float32(math.sqrt(T))).astype(NP_BF16)
    return cfq, cfk


_NC_CACHE = {}


def _get_nc(use_mask):
    key = bool(use_mask)
    if key not in _NC_CACHE:
        _NC_CACHE[key] = build_attention_nc(SEQ, HIDDEN, HG, HEAD_DIM, use_mask=key)
    return _NC_CACHE[key]


def _make_in_maps(input_ids, Wq, Wk, Wv, Wo, attention_mask=None):
    DG = HG * HEAD_DIM
    cfq, cfk = compute_cfacs(SEQ, HEAD_DIM)
    xb = [np.ascontiguousarray(input_ids[bi].T).astype(NP_BF16) for bi in range(BATCH)]
    wqb = Wq.astype(NP_BF16)
    wkb = Wk.astype(NP_BF16)
    wvb = Wv.astype(NP_BF16)
    wob = Wo.astype(NP_BF16)
    in_maps = []
    for core in range(N_CORES):
        bi, g = divmod(core, MP)
        m = {
            "xT": xb[bi],
            "wq": np.ascontiguousarray(wqb[:, g * DG : (g + 1) * DG]),
            "wk": np.ascontiguousarray(wkb[:, g * DG : (g + 1) * DG]),
            "wv": np.ascontiguousarray(wvb[:, g * DG : (g + 1) * DG]),
            "wo": np.ascontiguousarray(wob[g * DG : (g + 1) * DG, :]),
            "cfq": cfq,
            "cfk": cfk,
        }
        if attention_mask is not None:
            m["maskT"] = np.ascontiguousarray(attention_mask[bi, 0].T)
        in_maps.append(m)
    return in_maps


def prepare_for_bench(inputs):
    input_ids = np.asarray(inputs["input_ids"], dtype=np.float32)
    Wq = np.asarray(inputs["Wq"], dtype=np.float32)
    Wk = np.asarray(inputs["Wk"], dtype=np.float32)
    Wv = np.asarray(inputs["Wv"], dtype=np.float32)
    Wo = np.asarray(inputs["Wo"], dtype=np.float32)
    return _get_nc(False), _make_in_maps(input_ids, Wq, Wk, Wv, Wo)


def kernel(input_ids, attention_mask, Wq, Wk, Wv, Wo):
    input_ids = np.asarray(input_ids, dtype=np.float32)
    attention_mask = np.asarray(attention_mask, dtype=np.float32)
    Wq = np.asarray(Wq, dtype=np.float32)
    Wk = np.asarray(Wk, dtype=np.float32)
    Wv = np.asarray(Wv, dtype=np.float32)
    Wo = np.asarray(Wo, dtype=np.float32)

    b, t, c = input_ids.shape
    assert (b, t, c) == (BATCH, SEQ, HIDDEN)

    use_mask = bool(np.any(attention_mask))
    nc = _get_nc(use_mask)
    in_maps = _make_in_maps(
        input_ids, Wq, Wk, Wv, Wo, attention_mask if use_mask else None
    )

    res = bass_utils.run_bass_kernel_spmd(nc, in_maps, core_ids=list(range(N_CORES)))

    out = np.zeros((BATCH, SEQ, HIDDEN), dtype=np.float32)
    for bi in range(BATCH):
        acc = res.results[bi * MP]["out"].astype(np.float32)
        for g in range(1, MP):
            acc = acc + res.results[bi * MP + g]["out"].astype(np.float32)
        out[bi] = acc
    return out
